# revision 6
# baseline (speedup 1.0000x reference)
"""Trainium2 Bass kernel for DeepFunnelTransactionMLP.

MLP funnel 15->30->60->90->120->90->60->30->15->10->5->1 (ReLU between,
sigmoid at the end) over a batch of 524288 rows, fp32.

Strategy
--------
Pure data parallel: 8 cores x 65536 rows. On each core, activations are
kept feature-major (features on SBUF partitions, batch streaming on the
free dim), so every layer is one (or two) matmul(s) with the weight as
the stationary operand. Small layers are packed block-diagonally: e.g.
layer1 (15->30) processes 4 independent batch chunks in a single matmul
(4x15 input rows -> 4x30 output rows). Bias+ReLU are fused into single
ScalarE activation / VectorE tensor_scalar instructions reading PSUM.

Host side does the free work: transposing/packing x, building the
block-diagonal weights, and unpermuting the output.
"""

import os
import sys

# The bass PJRT path needs the axon jax platform; undo a cpu-only pin if one
# is set (harmless when jax was already imported by the caller).
if os.environ.get("JAX_PLATFORMS") not in (None, "", "axon", "axon,cpu"):
    os.environ["JAX_PLATFORMS"] = ""

sys.path.insert(0, "/opt/trn_rl_repo")

import numpy as np

import concourse.bacc as bacc
import concourse.mybir as mybir
from concourse.bass_utils import run_bass_kernel_spmd
from concourse.tile import TileContext

_DIMS = [15, 30, 60, 90, 120, 90, 60, 30, 15, 10, 5, 1]
NCORES = 8
B = 524288
BC = B // NCORES  # 65536 rows per core
S = 4096  # super-tile rows
NST = BC // S  # 16 super-tiles per core
F32 = mybir.dt.float32
F32R = mybir.dt.float32r

# Weight variants: (layer l (1-based), K, M, [(koff, moff), ...]).
# lhsT[koff+k, moff+m] = W_l[m, k] for each block; matmul out = lhsT.T @ rhs.
_VARIANTS = [
    ("w1", 1, 60, 120, [(15 * j, 30 * j) for j in range(4)]),
    ("w2A", 2, 60, 120, [(0, 0), (30, 60)]),
    ("w2B", 2, 120, 120, [(60, 0), (90, 60)]),
    ("w3A", 3, 60, 90, [(0, 0)]),
    ("w3B", 3, 120, 90, [(60, 0)]),
    ("w4", 4, 90, 120, [(0, 0)]),
    ("w5", 5, 120, 90, [(0, 0)]),
    ("w6A", 6, 90, 60, [(0, 0)]),
    ("w6B", 6, 90, 120, [(0, 60)]),
    ("w7A", 7, 120, 60, [(0, 0), (60, 30)]),
    ("w7B", 7, 120, 120, [(0, 60), (60, 90)]),
    ("w8A", 8, 120, 60, [(30 * j, 15 * j) for j in range(4)]),
    ("w8B", 8, 120, 120, [(30 * j, 60 + 15 * j) for j in range(4)]),
    # Tail layers stack into one shared PSUM tile at partition offsets
    # 0/80/120 (via moff), so L9+L10 share a single relu drain and L11 a
    # sigmoid drain over the same 512 columns.
    ("w9", 9, 120, 128, [(15 * j, 10 * j) for j in range(8)]),
    ("w10", 10, 80, 128, [(10 * j, 80 + 5 * j) for j in range(8)]),
    # w11 reads the TAIL ring from base partition 64 (PE requires rhs/lhsT
    # base in {0,32,64}); its blocks sit at +16 inside the [64:128] slice so
    # they line up with L10's outputs at partitions 80:120.
    ("w11", 11, 64, 128, [(80 + 5 * j, 120 + j) for j in range(8)]),
]
_VIDX = {name: i for i, (name, *_) in enumerate(_VARIANTS)}
# tight column packing: variant i starts at the cumulative sum of M widths
_WOFF = {}
_wc = 0
for name, _, _, M, _ in _VARIANTS:
    _WOFF[name] = _wc
    _wc += M
W_COLS = _wc

# Bias layouts: (layer l, tile count) -> packed [tile*dim] at column l-1.
_BIAS_TILES = [4, 2, 1, 1, 1, 2, 4, 8, 8, 8, 8]


def _pack_weights(Ws):
    w = np.zeros((128, W_COLS), dtype=np.float32)
    for name, l, K, M, blocks in _VARIANTS:
        Wl = Ws[l - 1]  # [fan_out, fan_in]
        fo, fi = Wl.shape
        c0 = _WOFF[name]
        for koff, moff in blocks:
            w[koff : koff + fi, c0 + moff : c0 + moff + fo] = Wl.T
    return w


def _pack_biases(bs):
    b = np.zeros((128, 16), dtype=np.float32)
    for l, (bl, nt) in enumerate(zip(bs, _BIAS_TILES)):
        v = np.tile(bl, nt)
        if l == 10:  # b11 sits at partitions 120:128 (tail psum stacking)
            b[120:128, l] = v
        else:
            b[: v.shape[0], l] = v
    # col 11: merged tail-drain bias: b9 x8 on parts 0:80, b10 x8 on 80:120
    b[0:80, 11] = np.tile(bs[8], 8)
    b[80:120, 11] = np.tile(bs[9], 8)
    return b


def _out_map():
    """batch-row (within a super-tile) for output element [group j, col n]."""
    M0 = np.arange(S).reshape(4, S // 4)
    M1 = M0
    M2 = np.empty((2, 2048), dtype=np.int64)
    for t in range(2):
        M2[:, 512 * t : 512 * (t + 1)] = M1[0:2, 512 * t : 512 * (t + 1)]
        M2[:, 1024 + 512 * t : 1024 + 512 * (t + 1)] = M1[2:4, 512 * t : 512 * (t + 1)]
    M3 = np.empty((1, 4096), dtype=np.int64)
    for u in range(4):
        M3[0, 512 * u : 512 * (u + 1)] = M2[0, 512 * u : 512 * (u + 1)]
        M3[0, 2048 + 512 * u : 2048 + 512 * (u + 1)] = M2[1, 512 * u : 512 * (u + 1)]
    M5 = M3
    M6 = np.empty((2, 2048), dtype=np.int64)
    for w in range(4):
        M6[0, 512 * w : 512 * (w + 1)] = M5[0, 1024 * w : 1024 * w + 512]
        M6[1, 512 * w : 512 * (w + 1)] = M5[0, 1024 * w + 512 : 1024 * w + 1024]
    M7 = np.empty((4, 1024), dtype=np.int64)
    for w in range(2):
        M7[0:2, 512 * w : 512 * (w + 1)] = M6[0:2, 1024 * w : 1024 * w + 512]
        M7[2:4, 512 * w : 512 * (w + 1)] = M6[0:2, 1024 * w + 512 : 1024 * w + 1024]
    M8 = np.empty((8, 512), dtype=np.int64)
    M8[0:4, :] = M7[0:4, 0:512]
    M8[4:8, :] = M7[0:4, 512:1024]
    return M8


_NC_CACHE = None


def _build_nc():
    global _NC_CACHE
    if _NC_CACHE is not None:
        return _NC_CACHE

    nc = bacc.Bacc("TRN2", target_bir_lowering=False, debug=False, num_devices=NCORES)
    xt = nc.dram_tensor("xt", [60, BC // 4], F32R, kind="ExternalInput")
    wd = nc.dram_tensor("w", [128, W_COLS], F32R, kind="ExternalInput")
    bd = nc.dram_tensor("b", [128, 16], F32, kind="ExternalInput")
    y = nc.dram_tensor("y", [8, BC // 8], F32, kind="ExternalOutput")

    with TileContext(nc) as tc:
        with (
            tc.tile_pool(name="const", bufs=1) as cpool,
            tc.tile_pool(name="act", bufs=1) as apool,
            tc.tile_pool(name="act2", bufs=2) as apool2,
            tc.tile_pool(name="io", bufs=3) as iopool,
            tc.tile_pool(name="psum", bufs=4, space="PSUM") as pspool,
        ):
            wsb = cpool.tile([128, W_COLS], F32R, tag="w")
            # TAIL ring: 2 slots of 512 cols; the merged tail drain writes
            # all 128 partitions each epoch, so every row w11's zero-padded
            # weights read is always initialized (no memset needed).
            tail = cpool.tile([128, 1024], F32R, tag="tail", name="tail")
            bsb = cpool.tile([128, 16], F32, tag="b")
            # Split the weight load so the first matmul (L1, needs cols
            # 0:360 = w1/w2A/w2B) isn't gated on the full 1.5MB transfer;
            # the remainder is queued behind ST0's xt DMA (see below).
            W_EARLY = 360
            nc.sync.dma_start(out=wsb[:, 0:W_EARLY], in_=wd[:, 0:W_EARLY])
            nc.sync.dma_start(out=bsb[:], in_=bd[:])
            late_w = [lambda: nc.sync.dma_start(out=wsb[:, W_EARLY:],
                                                in_=wd[:, W_EARLY:])]

            # Dummy sigmoid first: loads the sigmoid_and_others table set
            # (which also serves Relu) once during startup, instead of a
            # ~1.3us mid-pipeline table switch at the first real sigmoid.
            scr = cpool.tile([1, 1], F32, tag="scr", name="scr")
            nc.vector.memset(scr[:], 0.0)
            nc.scalar.activation(scr[:], scr[:],
                                 mybir.ActivationFunctionType.Sigmoid,
                                 bias=0.0, scale=1.0)

            def w_ap(name):
                _, _, K, M, _ = _VARIANTS[_VIDX[name]]
                c0 = _WOFF[name]
                if name == "w11":  # base-64 slice to match its rhs base
                    return wsb[64:128, c0 : c0 + M]
                return wsb[0:K, c0 : c0 + M]

            def b_ap(l, P):
                return bsb[0:P, l - 1 : l]

            # Greedy drain-engine balancing across ScalarE (1.2 cols/ns,
            # +185ns/instr), DVE (0.96, +125) and Pool/GpSimd (0.72, +95).
            # With the 10-deep pipeline below, every drain's consumer runs a
            # full epoch (~10us) later, so drain latency is irrelevant - only
            # aggregate engine busy matters. ScalarE pre-charged for the
            # forced per-ST sigmoids.
            eng_busy = [5500.0, 0.0, 0.0]

            def drain_cost(e, cols):
                # Only ScalarE and DVE can read PSUM on TRN2 (the BIR
                # verifier rejects GPSIMD-PSUM access), so drains have
                # exactly two lanes.
                if e == 0:
                    return cols / 1.2 + 185.0
                return cols / 0.96 + 125.0

            def drain(e, out_ap, in_ap, bias_ap):
                if e == 0:
                    nc.scalar.activation(
                        out_ap, in_ap, mybir.ActivationFunctionType.Relu,
                        bias=bias_ap, scale=1.0,
                    )
                else:
                    nc.vector.tensor_scalar(
                        out=out_ap, in0=in_ap,
                        scalar1=bias_ap, scalar2=0.0,
                        op0=mybir.AluOpType.add, op1=mybir.AluOpType.max,
                    )

            import os as _os
            _FORCE = _os.environ.get("DRAIN_FORCE")

            def pick(cols):
                if _FORCE is not None:
                    return int(_FORCE)
                e = min(range(2), key=lambda i: eng_busy[i] + drain_cost(i, cols))
                eng_busy[e] += drain_cost(e, cols)
                return e

            # Drains are deferred by one round: the drain for round k is
            # emitted after round k+1's matmuls, so when it reaches the head
            # of its engine's in-order queue its psum-stop semaphore is
            # already satisfied (no head-of-line blocking on the drain
            # engines). Consumers of the drained tile run a full epoch
            # later, so emission order still respects data dependencies.
            pending = []

            def defer(thunk):
                pending.append(thunk)

            def flush(keep=0):
                while len(pending) > keep:
                    pending.pop(0)()

            def round_(mms, out_tile, oc0, l, P, cols, eng_key=None):
                """One [128,1024] psum tile per round: 512-col matmul groups
                (start=True begins a group), then one fused bias+relu drain
                of the full round on the less-loaded PSUM-capable engine."""
                ps = pspool.tile([128, 1024], F32, tag="ps")
                q = -1
                for wname, rhs, start, stop in mms:
                    _, _, K, M, _ = _VARIANTS[_VIDX[wname]]
                    if start:
                        q += 1
                    nc.tensor.matmul(ps[0:M, 512 * q : 512 * q + 512], w_ap(wname),
                                     rhs, start=start, stop=stop)
                defer(lambda: drain(pick(cols), out_tile[0:P, oc0 : oc0 + cols],
                                    ps[0:P, 0:cols], b_ap(l, P)))

            def stage1(st, d):
                """DMA + L1, L2: 3 rounds."""
                rounds = []

                def r_dma():
                    c0 = (S // 4) * st
                    d["h0"] = iopool.tile([60, 1024], F32R, tag="h0", name="h0")
                    nc.sync.dma_start(out=d["h0"][:], in_=xt[:, c0 : c0 + 1024])
                    d["h1"] = apool2.tile([120, 1024], F32R, tag="h1", name="h1")
                    round_([("w1", d["h0"][0:60, 0:512], True, True),
                            ("w1", d["h0"][0:60, 512:1024], True, True)],
                           d["h1"], 0, 1, 120, 1024, ("L1", 0))
                rounds.append(r_dma)

                def r_l2(half):
                    def f():
                        if half == 0:
                            d["h2"] = apool2.tile([120, 2048], F32R, tag="h2", name="h2")
                            round_([("w2A", d["h1"][0:60, 0:512], True, True),
                                    ("w2A", d["h1"][0:60, 512:1024], True, True)],
                                   d["h2"], 0, 2, 120, 1024, ("L2", 0))
                        else:
                            round_([("w2B", d["h1"][0:120, 0:512], True, True),
                                    ("w2B", d["h1"][0:120, 512:1024], True, True)],
                                   d["h2"], 1024, 2, 120, 1024, ("L2", 1))
                    return f
                rounds += [r_l2(0), r_l2(1)]
                return rounds

            def stage2(st, d):
                """L3: 4 rounds."""
                def r_l3(r):
                    def f():
                        if r == 0:
                            d["h3"] = apool2.tile([90, 4096], F32R, tag="h3", name="h3")
                        if r < 2:
                            round_([("w3A", d["h2"][0:60, 1024 * r : 1024 * r + 512], True, True),
                                    ("w3A", d["h2"][0:60, 1024 * r + 512 : 1024 * (r + 1)], True, True)],
                                   d["h3"], 1024 * r, 3, 90, 1024, ("L3", r))
                        else:
                            rr = r - 2
                            round_([("w3B", d["h2"][0:120, 1024 * rr : 1024 * rr + 512], True, True),
                                    ("w3B", d["h2"][0:120, 1024 * rr + 512 : 1024 * (rr + 1)], True, True)],
                                   d["h3"], 2048 + 1024 * rr, 3, 90, 1024, ("L3", r))
                    return f
                return [r_l3(r) for r in range(4)]

            def stage3(st, d):
                """L4: 4 rounds."""
                def r_l4(r):
                    def f():
                        if r == 0:
                            d["h4"] = apool2.tile([120, 4096], F32R, tag="h4", name="h4")
                        round_([("w4", d["h3"][0:90, 1024 * r : 1024 * r + 512], True, True),
                                ("w4", d["h3"][0:90, 1024 * r + 512 : 1024 * (r + 1)], True, True)],
                               d["h4"], 1024 * r, 4, 120, 1024, ("L4", r))
                    return f
                return [r_l4(r) for r in range(4)]

            def stage4(st, d):
                """L5: 4 rounds."""
                def r_l5(r):
                    def f():
                        if r == 0:
                            d["h5"] = apool2.tile([90, 4096], F32R, tag="h5", name="h5")
                        round_([("w5", d["h4"][0:120, 1024 * r : 1024 * r + 512], True, True),
                                ("w5", d["h4"][0:120, 1024 * r + 512 : 1024 * (r + 1)], True, True)],
                               d["h5"], 1024 * r, 5, 90, 1024, ("L5", r))
                    return f
                return [r_l5(r) for r in range(4)]

            def stage5(st, d):
                """L6: 2 rounds."""
                def r_l6(r):
                    def f():
                        if r == 0:
                            d["h6"] = apool2.tile([120, 2048], F32R, tag="h6", name="h6")
                        mms = []
                        for q in range(2):
                            w = 2 * r + q
                            mms.append(("w6A", d["h5"][0:90, 1024 * w : 1024 * w + 512], True, False))
                            mms.append(("w6B", d["h5"][0:90, 1024 * w + 512 : 1024 * (w + 1)], False, True))
                        round_(mms, d["h6"], 1024 * r, 6, 120, 1024, ("L6", r))
                    return f
                return [r_l6(r) for r in range(2)]

            def stage6(st, d):
                """L7: 1 round."""
                def r_l7():
                    d["h7"] = apool2.tile([120, 1024], F32R, tag="h7", name="h7")
                    mms = []
                    for w in range(2):
                        mms.append(("w7A", d["h6"][0:120, 1024 * w : 1024 * w + 512], True, False))
                        mms.append(("w7B", d["h6"][0:120, 1024 * w + 512 : 1024 * (w + 1)], False, True))
                    round_(mms, d["h7"], 0, 7, 120, 1024, ("L7", 0))
                return [r_l7]

            # Tail stages share one [128,1024] psum tile per epoch: columns
            # 0:512 hold L8(st), columns 512:1024 stack L9(st-1) on parts
            # 0:80, L10(st-2) on 80:120 and L11(st-3) on 120:128 as one PE
            # accumulation group (zero-padded weights). One merged relu
            # drain [0:120] replaces L9+L10's two drains; L9/L10 outputs land
            # in a 2-slot resident ring (TAIL) indexed by epoch parity.
            # At fill/drain edge epochs not all of L9/L10/L11 are active, so
            # the group's alloc/start/stop/drain are epoch-aware: the first
            # active tail matmul starts (zeroes) the group, the last stops it
            # and emits the merged drain (+ sigmoid when L11 is active).
            tailps = {}

            def tail_slot(epoch):
                return tail[0:128, 512 * (epoch % 2) : 512 * (epoch % 2) + 512]

            def tail_ps(epoch):
                if epoch not in tailps:
                    tailps[epoch] = pspool.tile([128, 1024], F32, tag="ps",
                                                name="tailps")
                return tailps[epoch]

            def tail_stages_active(epoch):
                # which of (L9, L10, L11) run in this epoch
                return [k for k, off in ((9, 7), (10, 8), (11, 9))
                        if 0 <= epoch - off < NST]

            def tail_finish(epoch):
                """Merged relu drain for L9/L10 (valid parts only matter)."""
                ps = tailps.pop(epoch)
                defer(lambda: drain(pick(512), tail_slot(epoch)[0:128, :],
                                    ps[0:128, 512:1024], bsb[0:128, 11:12]))
                return ps

            def stage7(st, d):
                """L8: 1 round (also allocates the epoch's shared tail psum)."""
                def r_l8():
                    ps = tail_ps(st + 6)
                    d["h8"] = apool2.tile([120, 512], F32R, tag="h8", name="h8")
                    nc.tensor.matmul(ps[0:60, 0:512], w_ap("w8A"),
                                     d["h7"][0:120, 0:512], start=True, stop=False)
                    nc.tensor.matmul(ps[0:120, 0:512], w_ap("w8B"),
                                     d["h7"][0:120, 512:1024], start=False, stop=True)
                    defer(lambda: drain(pick(512), d["h8"][0:120, 0:512],
                                        ps[0:120, 0:512], b_ap(8, 120)))
                return [r_l8]

            def stage8(st, d):
                """L9 matmul into shared tail psum parts 0:80."""
                def r_l9():
                    epoch = st + 7
                    act = tail_stages_active(epoch)
                    nc.tensor.matmul(tail_ps(epoch)[0:128, 512:1024], w_ap("w9"),
                                     d["h8"][0:120, :],
                                     start=(act[0] == 9), stop=(act[-1] == 9))
                    if act[-1] == 9:
                        tail_finish(epoch)
                return [r_l9]

            def stage9(st, d):
                """L10 matmul into shared tail psum parts 80:120."""
                def r_l10():
                    epoch = st + 8
                    act = tail_stages_active(epoch)
                    nc.tensor.matmul(tail_ps(epoch)[0:128, 512:1024], w_ap("w10"),
                                     tail_slot(st + 7)[0:80, :],
                                     start=(act[0] == 10), stop=(act[-1] == 10))
                    if act[-1] == 10:
                        tail_finish(epoch)
                return [r_l10]

            def stage10(st, d):
                """L11 matmul (parts 120:128, group stop) + merged tail relu
                drain + sigmoid drain + output DMA: 1 round."""
                def r_l11():
                    epoch = st + 9
                    act = tail_stages_active(epoch)
                    nc.tensor.matmul(tail_ps(epoch)[0:128, 512:1024], w_ap("w11"),
                                     tail_slot(epoch - 1)[64:128, :],
                                     start=(act[0] == 11), stop=True)
                    ps = tail_finish(epoch)
                    # sigmoid for L11(st); output rides partitions 120:128.
                    # Engine APs need a 32-aligned base partition, so the
                    # drain covers [96:128] - rows 96:120 are junk sigmoids
                    # of L10 values and are never DMA'd out.
                    osb = iopool.tile([128, 512], F32, tag="osb", name="osb")
                    nc.scalar.activation(
                        osb[96:128, :], ps[96:128, 512:1024],
                        mybir.ActivationFunctionType.Sigmoid,
                        bias=bsb[96:128, 10:11], scale=1.0,
                    )
                    eng_busy[0] += drain_cost(0, 512)
                    nc.sync.dma_start(out=y[:, 512 * st : 512 * (st + 1)],
                                      in_=osb[120:128, :])
                return [r_l11]

            # 10-deep software pipeline: epoch e runs stage k on super-tile
            # e-k+1 round-robin. Every stage boundary is a layer boundary, so
            # each drain's consumer runs a full epoch (~10us) later: all sem
            # waits are satisfied long before the in-order sequencers reach
            # them, and the PE keeps a deep ready backlog. Tiles only live 2
            # epochs, so bufs=2 pools suffice regardless of pipeline depth.
            from itertools import zip_longest

            stages = [stage1, stage2, stage3, stage4, stage5, stage6,
                      stage7, stage8, stage9, stage10]
            dicts = [dict() for _ in range(NST)]
            for e in range(NST + len(stages) - 1):
                parts = []
                for i, b in enumerate(stages):
                    st = e - i
                    if 0 <= st < NST:
                        parts.append(b(st, dicts[st]))
                for grp in zip_longest(*parts):
                    for r in grp:
                        if r is not None:
                            r()
                            if late_w:
                                late_w.pop()()
                            flush(1)
            flush(0)

    nc.compile()
    _NC_CACHE = nc
    return nc


def _make_in_maps(inputs):
    x = np.asarray(inputs["x"], dtype=np.float32)
    Ws = [np.asarray(inputs[f"W{i}"], dtype=np.float32) for i in range(1, 12)]
    bs = [np.asarray(inputs[f"b{i}"], dtype=np.float32) for i in range(1, 12)]

    w_pack = _pack_weights(Ws)
    b_pack = _pack_biases(bs)

    in_maps = []
    for c in range(NCORES):
        xc = x[c * BC : (c + 1) * BC]
        # xt[15j+f, (S//4)*st + m] = xc[st*S + j*(S//4) + m, f]
        xt = np.ascontiguousarray(
            xc.reshape(NST, 4, S // 4, _DIMS[0]).transpose(1, 3, 0, 2).reshape(60, BC // 4)
        )
        in_maps.append({"xt": xt, "w": w_pack, "b": b_pack})
    return in_maps


def kernel(**inputs):
    in_maps = _make_in_maps(inputs)
    nc = _build_nc()
    res = run_bass_kernel_spmd(nc, in_maps, list(range(NCORES)))

    omap = _out_map()  # [8, 512] batch row within super-tile
    out = np.empty((B, 1), dtype=np.float32)
    for c in range(NCORES):
        yc = res.results[c]["y"]  # [8, BC//8]
        for st in range(NST):
            blk = np.empty(S, dtype=np.float32)
            blk[omap.ravel()] = yc[:, 512 * st : 512 * (st + 1)].ravel()
            out[c * BC + st * S : c * BC + (st + 1) * S, 0] = blk
    return out

